# revision 20
# baseline (speedup 1.0000x reference)
"""3-layer GAT on 8 Trainium2 NeuronCores (graph/data parallel by dst node).

v2 — gather-lean rework of the baseline:
  - Table rows are 256B pure-h bf16 (vs 512B): a_src/a_dst leave the row and
    arrive as ONE host-prepped per-edge stream esum = a_src[src] + a_dst[dst]
    (padding slots get -1e30 so w = exp(lrelu(esum)) underflows to exactly 0,
    replacing the gathered-`ones` zeroing trick).
  - Per-slot baked gather sizes: each core sorts its 49 dst blocks by edge
    count; program slot j takes the rank-j block of every core, with chunk
    counts KL[j]/KH[j] baked at compile time (vs one global max K). Rows
    gathered drop 131712 -> 110080 per core.
  - lo/hi table halves overlap in rows [17664, 32768); edges whose source row
    lands in the band are assigned to whichever half locally minimizes
    padding, so the int16-index split costs no extra chunks.
  - Gather calls are grouped 2 dst blocks per call (lo+hi per group) to
    amortize the ~1us fixed SWDGE descriptor-gen cost on GPSIMD.
  - one-hot is_equal runs fully in bf16 (iota/dstrel bf16) for DVE 2x mode;
    dense projections run with bf16 operands (f32 PSUM accumulate).
  - 4 SPMD launches as before: dense0 / edge0+dense1 / edge1+dense2 / edge2.
    Host work between launches: reassemble table slabs, build esum streams.
"""

import os
import sys
import copy
import types
import numpy as np
import ml_dtypes

if "/opt/trn_rl_repo" not in sys.path:
    sys.path.insert(0, "/opt/trn_rl_repo")

BF16 = ml_dtypes.bfloat16

N, E = 50000, 800000
NEG = 0.2

NCORES = 8
BLOCKS = 49                    # per core
NPC = BLOCKS * 128             # nodes per core = 6272
NPAD = NCORES * NPC            # 50176
TROWS = 50432                  # table rows: 0 = poison, position p -> row p+1
LO_LIM = 32768                 # lo half = rows [0, 32768)
HI_OFF = 17664                 # hi half = rows [17664, 50432)
HI_POISON = 50400 - HI_OFF     # all-zero junk row inside hi half
ROW = 128                      # bf16 words per table row (256B)
GRP = 2                        # dst blocks per gather-call group
MAXCH = 8                      # max 128-idx chunks per dma_gather call (9+ crashes the device)
ROWF = 64                      # f32 words per table row (256B; h bf16 bitcast)
PAD_NEG = -1.0e4               # esum padding: exp(lrelu(-1e4)) = exp(-2000) == 0.0


# --------------------------------------------------------------------------
# harness shims
# --------------------------------------------------------------------------
def _install_ntff_hook():
    """Register the NTFF profile hook the agent image's antenv lacks, so
    run_bass_kernel_spmd(trace=True) can report exec_time_ns."""
    try:
        import antenv
        if getattr(antenv, "axon_hooks", None) is not None:
            return True
        mod = types.ModuleType("antenv.axon_hooks")
        hook = [None]
        mod.set_axon_ntff_profile_hook = lambda h: hook.__setitem__(0, h)
        mod.get_axon_ntff_profile_hook = lambda: hook[0]
        antenv.axon_hooks = mod
        sys.modules["antenv.axon_hooks"] = mod
        from trn_agent_boot.trn_boot import _ntff_profile_via_ctypes
        mod.set_axon_ntff_profile_hook(
            _ntff_profile_via_ctypes("/opt/axon/libaxon_pjrt.so"))
        return hook[0] is not None
    except Exception:
        return False


def _split_multiwait_ctrl(nc, max_waits=1):
    """This walrus build rejects >1 semaphore wait on CTRL-class (Drain/Nop)
    instructions; split the TileContext tail drain into single-wait clones."""
    for bb in nc.main_func.blocks:
        newlist = []
        for ins in bb.instructions:
            si = ins.sync_info
            if (si is not None and si.on_wait and len(si.on_wait) > max_waits
                    and type(ins).__name__ in ("InstDrain", "InstNop")):
                waits = list(si.on_wait)
                si.on_wait = type(si.on_wait)([waits[0]])
                for i, w in enumerate(waits[1:]):
                    cl = copy.deepcopy(ins)
                    cl.name = f"{ins.name}-wsplit{i}"
                    cl.sync_info = copy.deepcopy(si)
                    cl.sync_info.on_wait = type(si.on_wait)([w])
                    cl.sync_info.on_update = type(si.on_update)([])
                    nc.register_instruction(cl, overwrite=True)
                    newlist.append(cl)
            newlist.append(ins)
        bb.instructions[:] = newlist


# --------------------------------------------------------------------------
# host-side graph prep (static per graph, layer-independent)
# --------------------------------------------------------------------------
def _wrap_idx(idx):
    """[ni] -> [128, ni//16] int16 in dma_gather's 16-partition wrapped
    layout, replicated to all 8 GPSIMD cores."""
    ni = idx.shape[0]
    w = np.zeros((16, ni // 16), dtype=np.int16)
    w[np.arange(ni) % 16, np.arange(ni) // 16] = idx
    return np.tile(w, (8, 1))


def _groups():
    return [list(range(g, min(g + GRP, BLOCKS))) for g in range(0, BLOCKS, GRP)]


def _prep_graph(edge_index):
    src = np.concatenate([np.asarray(edge_index[0], np.int64),
                          np.arange(N, dtype=np.int64)])
    dst = np.concatenate([np.asarray(edge_index[1], np.int64),
                          np.arange(N, dtype=np.int64)])
    core = dst // NPC
    blk = (dst % NPC) // 128

    # --- slot assignment: per core, rank blocks by total edge count --------
    tot = np.zeros((NCORES, BLOCKS), np.int64)
    np.add.at(tot, (core, blk), 1)
    order = np.argsort(-tot, axis=1, kind="stable")      # [c, slot] -> block
    invord = np.empty_like(order)
    for c in range(NCORES):
        invord[c, order[c]] = np.arange(BLOCKS)

    # table row of node n (and of padded nodes >= N)
    nodes = np.arange(NPAD, dtype=np.int64)
    ncore, nblk, nj = nodes // NPC, (nodes % NPC) // 128, nodes % 128
    rowof = 1 + ncore * NPC + invord[ncore, nblk] * 128 + nj   # [NPAD]

    srow = rowof[src]
    slot = invord[core, blk]
    gid = core * BLOCKS + slot                            # dst group id

    # --- lo/hi band assignment --------------------------------------------
    fl = srow < HI_OFF                                    # must be lo
    fh = srow >= LO_LIM                                   # must be hi
    cnt_fl = np.bincount(gid[fl], minlength=NCORES * BLOCKS).reshape(NCORES, BLOCKS)
    cnt_fh = np.bincount(gid[fh], minlength=NCORES * BLOCKS).reshape(NCORES, BLOCKS)
    tot_s = np.take_along_axis(tot, order, axis=1)        # [c, slot]

    KK = (tot_s + 127) // 128
    KL = np.maximum.reduce((cnt_fl + 127) // 128, axis=0)     # per-slot baked
    KH = np.maximum.reduce((cnt_fh + 127) // 128, axis=0)
    KKm = KK.max(axis=0)
    bump = np.maximum(KKm - (KL + KH), 0)
    KL = KL + bump
    # per (core, slot) lo quota: as many as fit, rest go hi
    lo_quota = np.minimum(cnt_fl + (tot_s - cnt_fl - cnt_fh), 128 * KL[None, :])
    lo_quota = np.maximum(lo_quota, tot_s - 128 * KH[None, :])
    assert (lo_quota >= cnt_fl).all() and (tot_s - lo_quota <= 128 * KH).all()

    # per-edge half: fl -> 0, fh -> 1, band -> first (quota - fixed) per group
    half = np.where(fh, 1, 0).astype(np.int64)
    band = ~fl & ~fh
    bo = np.argsort(gid[band] * 2, kind="stable")
    bidx = np.nonzero(band)[0][bo]                        # band edges by group
    gb = gid[bidx]
    gstart = np.searchsorted(gb, np.arange(NCORES * BLOCKS))
    gend = np.searchsorted(gb, np.arange(NCORES * BLOCKS) + 1)
    rank_in_g = np.arange(len(bidx)) - gstart[gb]
    quota_b = (lo_quota - cnt_fl).reshape(-1)[gb]
    half[bidx[rank_in_g >= quota_b]] = 1

    # --- bucket edges by (core, slot, half) -------------------------------
    key = gid * 2 + half
    eorder = np.argsort(key, kind="stable")
    ks = key[eorder]
    bounds = np.searchsorted(ks, np.arange(NCORES * BLOCKS * 2 + 1))

    groups = _groups()
    CH = int(KL.sum() + KH.sum())
    per_core = []
    for c in range(NCORES):
        gidx_parts = []
        calls = []
        dstrel = np.zeros((CH, 128), np.float32)
        eidx = np.full((CH, 128), -1, np.int64)
        cb = 0
        for gs in groups:
            for hf in (0, 1):
                idx_list = []
                for s in gs:
                    g2 = (c * BLOCKS + s) * 2 + hf
                    es = eorder[bounds[g2]:bounds[g2 + 1]]
                    es = es[np.argsort(srow[es], kind="stable")]
                    kcnt = int((KL if hf == 0 else KH)[s])
                    npad_ = kcnt * 128
                    idx = np.full(npad_, 0 if hf == 0 else HI_POISON, np.int64)
                    ne = len(es)
                    assert ne <= npad_
                    if ne:
                        r = srow[es]
                        idx[:ne] = r if hf == 0 else r - HI_OFF
                        dd = np.full(npad_, -1, np.int64)
                        dd[:ne] = es
                        eidx[cb:cb + kcnt] = dd.reshape(kcnt, 128)
                        dr = np.zeros(npad_, np.float32)
                        dr[:ne] = (dst[es] % 128).astype(np.float32)
                        dstrel[cb:cb + kcnt] = dr.reshape(kcnt, 128)
                    idx_list.append(idx)
                    cb += kcnt
                ii = np.concatenate(idx_list)
                assert (ii >= 0).all() and (ii < LO_LIM).all()
                for o in range(0, len(ii), MAXCH * 128):
                    part = ii[o:o + MAXCH * 128]
                    calls.append(len(part))
                    gidx_parts.append(_wrap_idx(part))
        assert cb == CH
        per_core.append(dict(
            gidx=np.ascontiguousarray(np.concatenate(gidx_parts, axis=1)),
            dstrel=np.ascontiguousarray(dstrel.T.astype(BF16)),
            eidx=eidx))
    return dict(KL=KL.astype(int), KH=KH.astype(int), CH=CH,
                calls=calls, per_core=per_core,
                rowof=rowof, src=src, dst=dst)


def _esum_stream(meta, c, arows, nh):
    """[128, CH*nh] f32: per-edge a_src[src]+a_dst[dst] (chunk, head) layout;
    padding slots get PAD_NEG so their weight underflows to zero."""
    eidx = meta["per_core"][c]["eidx"]                    # [CH, 128]
    CH = eidx.shape[0]
    ok = eidx >= 0
    e = eidx[ok]
    vals = np.full((CH, 128, nh), PAD_NEG, np.float32)
    vals[ok] = (arows[meta["src_row"][e], :nh]
                + arows[meta["dst_row"][e], 4:4 + nh])
    return np.ascontiguousarray(vals.transpose(1, 0, 2).reshape(128, CH * nh))


def _wext(W, a_s, a_d):
    """[128, 136] bf16 = [W | v_src | v_dst]; v_* = W @ att_* per head so
    a_src/a_dst fall out of the same dense matmul as h."""
    W = np.asarray(W, np.float32)
    a_s = np.asarray(a_s, np.float32)
    a_d = np.asarray(a_d, np.float32)
    heads, ch = a_s.shape
    out = np.zeros((128, 136), np.float32)
    out[:W.shape[0], :W.shape[1]] = W
    for h in range(heads):
        out[:W.shape[0], 128 + h] = W[:, h * ch:(h + 1) * ch] @ a_s[h]
        out[:W.shape[0], 132 + h] = W[:, h * ch:(h + 1) * ch] @ a_d[h]
    return out.astype(BF16)


# --------------------------------------------------------------------------
# device kernels
# --------------------------------------------------------------------------
_KER_CACHE = {}


def _get_kernels(meta):
    key = (tuple(meta["KL"]), tuple(meta["KH"]))
    if key not in _KER_CACHE:
        _KER_CACHE[key] = _build_kernels(meta)
    return _KER_CACHE[key]


def _build_kernels(meta):
    import concourse.mybir as mybir
    import concourse.tile as tile
    from concourse import bacc

    KL, KH, CH = meta["KL"], meta["KH"], meta["CH"]
    NIDX16 = sum(meta["calls"]) // 16
    dt = mybir.dt
    AF = mybir.ActivationFunctionType
    groups = _groups()

    def new_nc():
        return bacc.Bacc("TRN2", target_bir_lowering=False, debug=False,
                         num_swdge_queues=4)

    # ---- L0: dense only -------------------------------------------------
    nc0 = new_nc()
    xT = nc0.declare_dram_parameter("xT", [128, NPC], dt.bfloat16, False)
    w0 = nc0.declare_dram_parameter("wext", [128, 136], dt.bfloat16, False)
    slab_h = nc0.declare_dram_parameter("slab_h", [NPC, ROW], dt.bfloat16, True)
    slab_a = nc0.declare_dram_parameter("slab_a", [NPC, 8], dt.float32, True)
    with tile.TileContext(nc0) as tc:
        with tc.tile_pool(name="p", bufs=4) as pool, \
             tc.tile_pool(name="ps", bufs=4, space="PSUM") as pps:
            xts = pool.tile([128, NPC], dt.bfloat16, tag="xt")
            nc0.sync.dma_start(out=xts[:], in_=xT[:])
            ws = pool.tile([128, 136], dt.bfloat16, tag="w")
            nc0.sync.dma_start(out=ws[:], in_=w0[:])
            DB = 7              # blocks per batched output DMA
            for g0 in range(0, BLOCKS, DB):
                nb = min(DB, BLOCKS - g0)
                rowh = pool.tile([128, nb, ROW], dt.bfloat16, tag="rowh")
                rowa = pool.tile([128, nb, 8], dt.float32, tag="rowa")
                for i in range(nb):
                    b = g0 + i
                    ps = pps.tile([128, 136], dt.float32, tag="h")
                    nc0.tensor.matmul(ps[:], lhsT=xts[:, b * 128:(b + 1) * 128],
                                      rhs=ws[:], start=True, stop=True)
                    nc0.scalar.activation(rowh[:, i, :], ps[:, 0:128], AF.Copy)
                    nc0.vector.tensor_copy(rowa[:, i, :], ps[:, 128:136])
                nc0.sync.dma_start(
                    out=slab_h[g0 * 128:(g0 + nb) * 128, :]
                        .rearrange("(b p) r -> p b r", p=128),
                    in_=rowh[:])
                nc0.sync.dma_start(
                    out=slab_a[g0 * 128:(g0 + nb) * 128, :]
                        .rearrange("(b p) r -> p b r", p=128),
                    in_=rowa[:])
    _split_multiwait_ctrl(nc0)
    nc0.compile()

    # ---- edge phase (+ optional fused next dense) -----------------------
    def build_edge(last):
        nc = new_nc()
        table = nc.declare_dram_parameter("table", [TROWS, ROWF], dt.float32, False)
        gidx = nc.declare_dram_parameter("gidx", [128, NIDX16], dt.int16, False)
        dstrel = nc.declare_dram_parameter("dstrel", [128, CH], dt.bfloat16, False)
        NH = 1 if last else 4
        HC = 64 if last else 128
        MC = HC + NH
        C = HC // NH
        esum = nc.declare_dram_parameter("esum", [128, CH * NH], dt.float32, False)
        iota = nc.declare_dram_parameter("iota", [128, 128], dt.bfloat16, False)
        bias = nc.declare_dram_parameter("bias", [128, HC], dt.float32, False)
        if last:
            out = nc.declare_dram_parameter("out", [NPC, HC], dt.float32, True)
        else:
            ident = nc.declare_dram_parameter("ident", [128, 128], dt.float32, False)
            wnext = nc.declare_dram_parameter("wext", [128, 136], dt.bfloat16, False)
            out_h = nc.declare_dram_parameter("slab_h", [NPC, ROW], dt.bfloat16, True)
            out_a = nc.declare_dram_parameter("slab_a", [NPC, 8], dt.float32, True)

        with tile.TileContext(nc) as tc:
            with tc.tile_pool(name="c", bufs=1) as cpool, \
                 tc.tile_pool(name="g", bufs=3) as gpool, \
                 tc.tile_pool(name="w", bufs=3) as wpool, \
                 tc.tile_pool(name="ps", bufs=2, space="PSUM") as pps, \
                 tc.tile_pool(name="ps2", bufs=2, space="PSUM") as pps2:
                regs = {}
                for ni in meta["calls"]:
                    if ni not in regs:
                        regs[ni] = nc.gpsimd.to_reg(ni)
                iot = cpool.tile([128, 128], dt.bfloat16, tag="iota")
                nc.sync.dma_start(out=iot[:], in_=iota[:])
                bia = cpool.tile([128, HC], dt.float32, tag="bias")
                nc.sync.dma_start(out=bia[:], in_=bias[:])
                idxs = cpool.tile([128, NIDX16], dt.int16, tag="gidx")
                nc.sync.dma_start(out=idxs[:], in_=gidx[:])
                drel = cpool.tile([128, CH], dt.bfloat16, tag="drel")
                nc.sync.dma_start(out=drel[:], in_=dstrel[:])
                esu = cpool.tile([128, CH * NH], dt.float32, tag="esum")
                nc.sync.dma_start(out=esu[:], in_=esum[:])
                if not last:
                    idn = cpool.tile([128, 128], dt.float32, tag="ident")
                    nc.sync.dma_start(out=idn[:], in_=ident[:])
                    wnx = cpool.tile([128, 136], dt.bfloat16, tag="wext")
                    nc.sync.dma_start(out=wnx[:], in_=wnext[:])

                tab_lo = table[0:LO_LIM, :]
                tab_hi = table[HI_OFF:TROWS, :]
                ioff = 0
                qn = 0
                ci = 0          # call index
                cb = 0          # chunk base
                for gs in groups:
                    kls = [int(KL[s]) for s in gs]
                    khs = [int(KH[s]) for s in gs]
                    KG = sum(kls) + sum(khs)
                    G = gpool.tile([128, KG, ROWF], dt.float32, tag="G")
                    Gb = G[:].bitcast(dt.bfloat16)      # [128, KG, 128]
                    k0 = 0
                    for hf, kk in ((0, sum(kls)), (1, sum(khs))):
                        for o in range(0, kk, MAXCH):
                            kc = min(MAXCH, kk - o)
                            ni = kc * 128
                            assert ni == meta["calls"][ci]
                            nc.gpsimd.dma_gather(
                                G[:, k0:k0 + kc, :],
                                tab_lo if hf == 0 else tab_hi,
                                idxs[:, ioff:ioff + ni // 16],
                                num_idxs=ni, num_idxs_reg=regs[ni],
                                elem_size=ROWF, queue_num=qn)
                            qn = (qn + 1) % 4
                            ioff += ni // 16
                            k0 += kc
                            ci += 1

                    # w = exp(lrelu(esum)); padding slots underflow to 0
                    wv = wpool.tile([128, KG * NH], dt.float32, tag="wv")
                    nc.scalar.activation(wv[:], esu[:, cb * NH:(cb + KG) * NH],
                                         AF.Prelu, alpha=NEG)
                    nc.scalar.activation(wv[:], wv[:], AF.Exp)

                    # one-hot(dst_rel) [128, KG, 128] bf16
                    oh = wpool.tile([128, KG * 128], dt.bfloat16, tag="oh")
                    nc.vector.tensor_tensor(
                        oh[:].rearrange("p (k j) -> p k j", j=128),
                        drel[:, cb:cb + KG]
                            .rearrange("p (k o) -> p k o", o=1)
                            .to_broadcast([128, KG, 128]),
                        iot[:].rearrange("p (o j) -> p o j", o=1)
                            .to_broadcast([128, KG, 128]),
                        op=mybir.AluOpType.is_equal)

                    # M = [h*w | w] bf16; w lands in M's tail columns via the
                    # scalar-engine bf16 cast and doubles as the mul operand
                    M = wpool.tile([128, KG * MC], dt.bfloat16, tag="M")
                    Mv = M[:].rearrange("p (k m) -> p k m", m=MC)
                    nc.scalar.activation(
                        Mv[:, :, HC:MC],
                        wv[:].rearrange("p (k h) -> p k h", h=NH), AF.Copy)
                    nc.vector.tensor_mul(
                        Mv[:, :, 0:HC].rearrange("p k (h c) -> p k h c", c=C),
                        Gb[:, :, 0:HC].rearrange("p k (h c) -> p k h c", c=C),
                        Mv[:, :, HC:MC].rearrange("p k (h o) -> p k h o", o=1)
                            .to_broadcast([128, KG, NH, C]))

                    # per-slot scatter-accumulate + epilogue
                    ng = len(gs)
                    if last:
                        oxp = wpool.tile([128, ng, HC], dt.float32, tag="oxp")
                    else:
                        growh = wpool.tile([128, ng, ROW], dt.bfloat16,
                                           tag="growh")
                        growa = wpool.tile([128, ng, 8], dt.float32, tag="growa")
                    kbase_lo = 0
                    kbase_hi = sum(kls)
                    for i, s in enumerate(gs):
                        chunks = (list(range(kbase_lo, kbase_lo + kls[i]))
                                  + list(range(kbase_hi, kbase_hi + khs[i])))
                        kbase_lo += kls[i]
                        kbase_hi += khs[i]
                        T = pps.tile([128, MC], dt.float32, tag=f"acc{i}")
                        for j, k in enumerate(chunks):
                            nc.tensor.matmul(T[:],
                                             lhsT=oh[:, k * 128:(k + 1) * 128],
                                             rhs=Mv[:, k, :],
                                             start=(j == 0),
                                             stop=(j == len(chunks) - 1))
                        rcp = wpool.tile([128, NH], dt.float32, tag=f"rcp{i}")
                        nc.vector.reciprocal(rcp[:], T[:, HC:MC])
                        xp = wpool.tile([128, HC], dt.float32, tag=f"xp{i}")
                        nc.vector.tensor_mul(
                            xp[:].rearrange("p (h c) -> p h c", c=C),
                            T[:, 0:HC].rearrange("p (h c) -> p h c", c=C),
                            rcp[:].rearrange("p (h o) -> p h o", o=1)
                                .to_broadcast([128, NH, C]))
                        nc.vector.tensor_add(xp[:], xp[:], bia[:])
                        if last:
                            nc.scalar.activation(oxp[:, i, :], xp[:],
                                                 AF.Prelu, alpha=NEG)
                        else:
                            nc.scalar.activation(xp[:], xp[:], AF.Prelu,
                                                 alpha=NEG)
                            pt = pps2.tile([128, 128], dt.float32, tag="xt")
                            nc.tensor.transpose(out=pt[:], in_=xp[:],
                                                identity=idn[:])
                            xt = wpool.tile([128, 128], dt.bfloat16,
                                            tag=f"xts{i}")
                            nc.scalar.activation(xt[:], pt[:], AF.Copy)
                            ph = pps2.tile([128, 136], dt.float32, tag="h2")
                            nc.tensor.matmul(ph[:], lhsT=xt[:], rhs=wnx[:],
                                             start=True, stop=True)
                            nc.scalar.activation(growh[:, i, :], ph[:, 0:128],
                                                 AF.Copy)
                            nc.vector.tensor_copy(growa[:, i, :], ph[:, 128:136])
                    s0 = gs[0]
                    s1 = gs[-1] + 1
                    if last:
                        nc.sync.dma_start(
                            out=out[s0 * 128:s1 * 128, :]
                                .rearrange("(b p) r -> p b r", p=128),
                            in_=oxp[:])
                    else:
                        nc.sync.dma_start(
                            out=out_h[s0 * 128:s1 * 128, :]
                                .rearrange("(b p) r -> p b r", p=128),
                            in_=growh[:])
                        nc.sync.dma_start(
                            out=out_a[s0 * 128:s1 * 128, :]
                                .rearrange("(b p) r -> p b r", p=128),
                            in_=growa[:])
                    cb += KG
        _split_multiwait_ctrl(nc)
        nc.compile()
        return nc

    return nc0, build_edge(False), build_edge(True)


# --------------------------------------------------------------------------
# entry point
# --------------------------------------------------------------------------
def kernel(x, edge_index, W0, as0, ad0, b0, W1, as1, ad1, b1, W2, as2, ad2, b2):
    _install_ntff_hook()
    from concourse.bass_utils import run_bass_kernel_spmd

    x = np.asarray(x, np.float32)
    meta = _prep_graph(np.asarray(edge_index))
    meta["src_row"] = meta["rowof"][meta["src"]]
    meta["dst_row"] = meta["rowof"][meta["dst"]]
    nc0, nc12, nc3 = _get_kernels(meta)
    cores = list(range(NCORES))
    trace = bool(os.environ.get("BASS_TRACE"))

    iota = np.tile(np.arange(128, dtype=np.float32), (128, 1)).astype(BF16)
    ident = np.eye(128, dtype=np.float32)
    w0e, w1e, w2e = _wext(W0, as0, ad0), _wext(W1, as1, ad1), _wext(W2, as2, ad2)
    rowof = meta["rowof"]
    garbage_rows = rowof[N:]

    total_ns = [0]

    def run(nc, maps):
        last = None
        for attempt in range(3):
            try:
                r = run_bass_kernel_spmd(nc, maps, core_ids=cores, trace=trace)
                if r.exec_time_ns:
                    total_ns[0] += int(r.exec_time_ns)
                    if os.environ.get("KERNEL_VERBOSE"):
                        print(f"[launch] exec={r.exec_time_ns}ns", file=sys.stderr)
                return r.results
            except Exception as e:  # intermittent NRT exec-unit crashes
                last = e
        raise last

    def assemble(res):
        th = np.zeros((TROWS, ROW), BF16)
        ar = np.zeros((TROWS, 8), np.float32)
        th[1:NPAD + 1] = np.concatenate(
            [np.asarray(res[c]["slab_h"]) for c in cores], axis=0)
        ar[1:NPAD + 1] = np.concatenate(
            [np.asarray(res[c]["slab_a"]) for c in cores], axis=0)
        th[garbage_rows] = BF16(0)
        ar[garbage_rows] = 0.0
        return np.ascontiguousarray(th).view(np.float32), ar

    # dense0: x rows permuted into table order
    perm_nodes = np.empty(NPAD, np.int64)
    perm_nodes[rowof - 1] = np.arange(NPAD)
    xpad = np.zeros((NPAD, 128), np.float32)
    xpad[:N] = x
    xTb = np.ascontiguousarray(xpad[perm_nodes].T.astype(BF16))
    res = run(nc0, [{"xT": np.ascontiguousarray(xTb[:, c * NPC:(c + 1) * NPC]),
                     "wext": w0e} for c in cores])
    table, arows = assemble(res)

    def edge_maps(tab, ar, wnext, bias_vec, hc, nh):
        bias = np.tile(np.asarray(bias_vec, np.float32)[:hc], (128, 1))
        maps = []
        for c in cores:
            pc = meta["per_core"][c]
            m = {"table": tab, "gidx": pc["gidx"], "dstrel": pc["dstrel"],
                 "esum": _esum_stream(meta, c, ar, nh),
                 "iota": iota, "bias": bias}
            if wnext is not None:
                m["ident"] = ident
                m["wext"] = wnext
            maps.append(m)
        return maps

    res = run(nc12, edge_maps(table, arows, w1e, b0, 128, 4))
    table, arows = assemble(res)
    res = run(nc12, edge_maps(table, arows, w2e, b1, 128, 4))
    table, arows = assemble(res)
    res = run(nc3, edge_maps(table, arows, None, b2, 64, 1))
    outp = np.concatenate([np.asarray(res[c]["out"]) for c in cores], axis=0)
    out = np.empty((NPAD, 64), np.float32)
    out[perm_nodes] = outp
    kernel.last_exec_ns = total_ns[0]
    return np.ascontiguousarray(out[:N], dtype=np.float32)


# revision 21
# speedup vs baseline: 1.0167x; 1.0167x over previous
"""3-layer GAT on 8 Trainium2 NeuronCores (graph/data parallel by dst node).

v2 — gather-lean rework of the baseline:
  - Table rows are 256B pure-h bf16 (vs 512B): a_src/a_dst leave the row and
    arrive as ONE host-prepped per-edge stream esum = a_src[src] + a_dst[dst]
    (padding slots get -1e30 so w = exp(lrelu(esum)) underflows to exactly 0,
    replacing the gathered-`ones` zeroing trick).
  - Per-slot baked gather sizes: each core sorts its 49 dst blocks by edge
    count; program slot j takes the rank-j block of every core, with chunk
    counts KL[j]/KH[j] baked at compile time (vs one global max K). Rows
    gathered drop 131712 -> 110080 per core.
  - lo/hi table halves overlap in rows [17664, 32768); edges whose source row
    lands in the band are assigned to whichever half locally minimizes
    padding, so the int16-index split costs no extra chunks.
  - Gather calls are grouped 2 dst blocks per call (lo+hi per group) to
    amortize the ~1us fixed SWDGE descriptor-gen cost on GPSIMD.
  - one-hot is_equal runs fully in bf16 (iota/dstrel bf16) for DVE 2x mode;
    dense projections run with bf16 operands (f32 PSUM accumulate).
  - 4 SPMD launches as before: dense0 / edge0+dense1 / edge1+dense2 / edge2.
    Host work between launches: reassemble table slabs, build esum streams.
"""

import os
import sys
import copy
import types
import numpy as np
import ml_dtypes

if "/opt/trn_rl_repo" not in sys.path:
    sys.path.insert(0, "/opt/trn_rl_repo")

BF16 = ml_dtypes.bfloat16

N, E = 50000, 800000
NEG = 0.2

NCORES = 8
BLOCKS = 49                    # per core
NPC = BLOCKS * 128             # nodes per core = 6272
NPAD = NCORES * NPC            # 50176
TROWS = 50432                  # table rows: 0 = poison, position p -> row p+1
LO_LIM = 32768                 # lo half = rows [0, 32768)
HI_OFF = 17664                 # hi half = rows [17664, 50432)
HI_POISON = 50400 - HI_OFF     # all-zero junk row inside hi half
ROW = 128                      # bf16 words per table row (256B)
GRP = 2                        # dst blocks per gather-call group
MAXCH = 8                      # max 128-idx chunks per dma_gather call (9+ crashes the device)
ROWF = 64                      # f32 words per table row (256B; h bf16 bitcast)
PAD_NEG = -1.0e4               # esum padding: exp(lrelu(-1e4)) = exp(-2000) == 0.0


# --------------------------------------------------------------------------
# harness shims
# --------------------------------------------------------------------------
def _install_ntff_hook():
    """Register the NTFF profile hook the agent image's antenv lacks, so
    run_bass_kernel_spmd(trace=True) can report exec_time_ns."""
    try:
        import antenv
        if getattr(antenv, "axon_hooks", None) is not None:
            return True
        mod = types.ModuleType("antenv.axon_hooks")
        hook = [None]
        mod.set_axon_ntff_profile_hook = lambda h: hook.__setitem__(0, h)
        mod.get_axon_ntff_profile_hook = lambda: hook[0]
        antenv.axon_hooks = mod
        sys.modules["antenv.axon_hooks"] = mod
        from trn_agent_boot.trn_boot import _ntff_profile_via_ctypes
        mod.set_axon_ntff_profile_hook(
            _ntff_profile_via_ctypes("/opt/axon/libaxon_pjrt.so"))
        return hook[0] is not None
    except Exception:
        return False


def _split_multiwait_ctrl(nc, max_waits=1):
    """This walrus build rejects >1 semaphore wait on CTRL-class (Drain/Nop)
    instructions; split the TileContext tail drain into single-wait clones."""
    for bb in nc.main_func.blocks:
        newlist = []
        for ins in bb.instructions:
            si = ins.sync_info
            if (si is not None and si.on_wait and len(si.on_wait) > max_waits
                    and type(ins).__name__ in ("InstDrain", "InstNop")):
                waits = list(si.on_wait)
                si.on_wait = type(si.on_wait)([waits[0]])
                for i, w in enumerate(waits[1:]):
                    cl = copy.deepcopy(ins)
                    cl.name = f"{ins.name}-wsplit{i}"
                    cl.sync_info = copy.deepcopy(si)
                    cl.sync_info.on_wait = type(si.on_wait)([w])
                    cl.sync_info.on_update = type(si.on_update)([])
                    nc.register_instruction(cl, overwrite=True)
                    newlist.append(cl)
            newlist.append(ins)
        bb.instructions[:] = newlist


# --------------------------------------------------------------------------
# host-side graph prep (static per graph, layer-independent)
# --------------------------------------------------------------------------
def _wrap_idx(idx):
    """[ni] -> [128, ni//16] int16 in dma_gather's 16-partition wrapped
    layout, replicated to all 8 GPSIMD cores."""
    ni = idx.shape[0]
    w = np.zeros((16, ni // 16), dtype=np.int16)
    w[np.arange(ni) % 16, np.arange(ni) // 16] = idx
    return np.tile(w, (8, 1))


def _groups():
    return [list(range(g, min(g + GRP, BLOCKS))) for g in range(0, BLOCKS, GRP)]


def _prep_graph(edge_index):
    src = np.concatenate([np.asarray(edge_index[0], np.int64),
                          np.arange(N, dtype=np.int64)])
    dst = np.concatenate([np.asarray(edge_index[1], np.int64),
                          np.arange(N, dtype=np.int64)])
    core = dst // NPC
    blk = (dst % NPC) // 128

    # --- slot assignment: per core, rank blocks by total edge count --------
    tot = np.zeros((NCORES, BLOCKS), np.int64)
    np.add.at(tot, (core, blk), 1)
    order = np.argsort(-tot, axis=1, kind="stable")      # [c, slot] -> block
    invord = np.empty_like(order)
    for c in range(NCORES):
        invord[c, order[c]] = np.arange(BLOCKS)

    # table row of node n (and of padded nodes >= N)
    nodes = np.arange(NPAD, dtype=np.int64)
    ncore, nblk, nj = nodes // NPC, (nodes % NPC) // 128, nodes % 128
    rowof = 1 + ncore * NPC + invord[ncore, nblk] * 128 + nj   # [NPAD]

    srow = rowof[src]
    slot = invord[core, blk]
    gid = core * BLOCKS + slot                            # dst group id

    # --- lo/hi band assignment --------------------------------------------
    fl = srow < HI_OFF                                    # must be lo
    fh = srow >= LO_LIM                                   # must be hi
    cnt_fl = np.bincount(gid[fl], minlength=NCORES * BLOCKS).reshape(NCORES, BLOCKS)
    cnt_fh = np.bincount(gid[fh], minlength=NCORES * BLOCKS).reshape(NCORES, BLOCKS)
    tot_s = np.take_along_axis(tot, order, axis=1)        # [c, slot]

    KK = (tot_s + 127) // 128
    KL = np.maximum.reduce((cnt_fl + 127) // 128, axis=0)     # per-slot baked
    KH = np.maximum.reduce((cnt_fh + 127) // 128, axis=0)
    KKm = KK.max(axis=0)
    bump = np.maximum(KKm - (KL + KH), 0)
    KL = KL + bump
    # per (core, slot) lo quota: as many as fit, rest go hi
    lo_quota = np.minimum(cnt_fl + (tot_s - cnt_fl - cnt_fh), 128 * KL[None, :])
    lo_quota = np.maximum(lo_quota, tot_s - 128 * KH[None, :])
    assert (lo_quota >= cnt_fl).all() and (tot_s - lo_quota <= 128 * KH).all()

    # per-edge half: fl -> 0, fh -> 1, band -> first (quota - fixed) per group
    half = np.where(fh, 1, 0).astype(np.int64)
    band = ~fl & ~fh
    bo = np.argsort(gid[band] * 2, kind="stable")
    bidx = np.nonzero(band)[0][bo]                        # band edges by group
    gb = gid[bidx]
    gstart = np.searchsorted(gb, np.arange(NCORES * BLOCKS))
    gend = np.searchsorted(gb, np.arange(NCORES * BLOCKS) + 1)
    rank_in_g = np.arange(len(bidx)) - gstart[gb]
    quota_b = (lo_quota - cnt_fl).reshape(-1)[gb]
    half[bidx[rank_in_g >= quota_b]] = 1

    # --- bucket edges by (core, slot, half) -------------------------------
    key = gid * 2 + half
    eorder = np.argsort(key, kind="stable")
    ks = key[eorder]
    bounds = np.searchsorted(ks, np.arange(NCORES * BLOCKS * 2 + 1))

    groups = _groups()
    CH = int(KL.sum() + KH.sum())
    per_core = []
    for c in range(NCORES):
        gidx_parts = []
        calls = []
        dstrel = np.zeros((CH, 128), np.float32)
        eidx = np.full((CH, 128), -1, np.int64)
        cb = 0
        for gs in groups:
            for hf in (0, 1):
                idx_list = []
                for s in gs:
                    g2 = (c * BLOCKS + s) * 2 + hf
                    es = eorder[bounds[g2]:bounds[g2 + 1]]
                    es = es[np.argsort(srow[es], kind="stable")]
                    kcnt = int((KL if hf == 0 else KH)[s])
                    npad_ = kcnt * 128
                    idx = np.full(npad_, 0 if hf == 0 else HI_POISON, np.int64)
                    ne = len(es)
                    assert ne <= npad_
                    if ne:
                        r = srow[es]
                        idx[:ne] = r if hf == 0 else r - HI_OFF
                        dd = np.full(npad_, -1, np.int64)
                        dd[:ne] = es
                        eidx[cb:cb + kcnt] = dd.reshape(kcnt, 128)
                        dr = np.zeros(npad_, np.float32)
                        dr[:ne] = (dst[es] % 128).astype(np.float32)
                        dstrel[cb:cb + kcnt] = dr.reshape(kcnt, 128)
                    idx_list.append(idx)
                    cb += kcnt
                ii = np.concatenate(idx_list)
                assert (ii >= 0).all() and (ii < LO_LIM).all()
                for o in range(0, len(ii), MAXCH * 128):
                    part = ii[o:o + MAXCH * 128]
                    calls.append(len(part))
                    gidx_parts.append(_wrap_idx(part))
        assert cb == CH
        per_core.append(dict(
            gidx=np.ascontiguousarray(np.concatenate(gidx_parts, axis=1)),
            dstrel=np.ascontiguousarray(dstrel.T.astype(BF16)),
            eidx=eidx))
    return dict(KL=KL.astype(int), KH=KH.astype(int), CH=CH,
                calls=calls, per_core=per_core,
                rowof=rowof, src=src, dst=dst)


def _esum_stream(meta, c, arows, nh):
    """[128, CH*nh] f32: per-edge a_src[src]+a_dst[dst] (chunk, head) layout;
    padding slots get PAD_NEG so their weight underflows to zero."""
    eidx = meta["per_core"][c]["eidx"]                    # [CH, 128]
    CH = eidx.shape[0]
    ok = eidx >= 0
    e = eidx[ok]
    vals = np.full((CH, 128, nh), PAD_NEG, np.float32)
    vals[ok] = (arows[meta["src_row"][e], :nh]
                + arows[meta["dst_row"][e], 4:4 + nh])
    return np.ascontiguousarray(vals.transpose(1, 0, 2).reshape(128, CH * nh))


def _wext(W, a_s, a_d):
    """[128, 136] bf16 = [W | v_src | v_dst]; v_* = W @ att_* per head so
    a_src/a_dst fall out of the same dense matmul as h."""
    W = np.asarray(W, np.float32)
    a_s = np.asarray(a_s, np.float32)
    a_d = np.asarray(a_d, np.float32)
    heads, ch = a_s.shape
    out = np.zeros((128, 136), np.float32)
    out[:W.shape[0], :W.shape[1]] = W
    for h in range(heads):
        out[:W.shape[0], 128 + h] = W[:, h * ch:(h + 1) * ch] @ a_s[h]
        out[:W.shape[0], 132 + h] = W[:, h * ch:(h + 1) * ch] @ a_d[h]
    return out.astype(BF16)


# --------------------------------------------------------------------------
# device kernels
# --------------------------------------------------------------------------
_KER_CACHE = {}


def _get_kernels(meta):
    key = (tuple(meta["KL"]), tuple(meta["KH"]))
    if key not in _KER_CACHE:
        _KER_CACHE[key] = _build_kernels(meta)
    return _KER_CACHE[key]


def _build_kernels(meta):
    import concourse.mybir as mybir
    import concourse.tile as tile
    from concourse import bacc

    KL, KH, CH = meta["KL"], meta["KH"], meta["CH"]
    NIDX16 = sum(meta["calls"]) // 16
    dt = mybir.dt
    AF = mybir.ActivationFunctionType
    groups = _groups()

    def new_nc():
        return bacc.Bacc("TRN2", target_bir_lowering=False, debug=False,
                         num_swdge_queues=4)

    # ---- L0: dense only -------------------------------------------------
    nc0 = new_nc()
    xT = nc0.declare_dram_parameter("xT", [128, NPC], dt.bfloat16, False)
    w0 = nc0.declare_dram_parameter("wext", [128, 136], dt.bfloat16, False)
    slab_h = nc0.declare_dram_parameter("slab_h", [NPC, ROW], dt.bfloat16, True)
    slab_a = nc0.declare_dram_parameter("slab_a", [NPC, 8], dt.float32, True)
    with tile.TileContext(nc0) as tc:
        with tc.tile_pool(name="p", bufs=4) as pool, \
             tc.tile_pool(name="ps", bufs=4, space="PSUM") as pps:
            xts = pool.tile([128, NPC], dt.bfloat16, tag="xt")
            nc0.sync.dma_start(out=xts[:], in_=xT[:])
            ws = pool.tile([128, 136], dt.bfloat16, tag="w")
            nc0.sync.dma_start(out=ws[:], in_=w0[:])
            DB = 7              # blocks per batched output DMA
            for g0 in range(0, BLOCKS, DB):
                nb = min(DB, BLOCKS - g0)
                rowh = pool.tile([128, nb, ROW], dt.bfloat16, tag="rowh")
                rowa = pool.tile([128, nb, 8], dt.float32, tag="rowa")
                for i in range(nb):
                    b = g0 + i
                    ps = pps.tile([128, 136], dt.float32, tag="h")
                    nc0.tensor.matmul(ps[:], lhsT=xts[:, b * 128:(b + 1) * 128],
                                      rhs=ws[:], start=True, stop=True)
                    nc0.scalar.activation(rowh[:, i, :], ps[:, 0:128], AF.Copy)
                    nc0.vector.tensor_copy(rowa[:, i, :], ps[:, 128:136])
                nc0.sync.dma_start(
                    out=slab_h[g0 * 128:(g0 + nb) * 128, :]
                        .rearrange("(b p) r -> p b r", p=128),
                    in_=rowh[:])
                nc0.sync.dma_start(
                    out=slab_a[g0 * 128:(g0 + nb) * 128, :]
                        .rearrange("(b p) r -> p b r", p=128),
                    in_=rowa[:])
    _split_multiwait_ctrl(nc0)
    nc0.compile()

    # ---- edge phase (+ optional fused next dense) -----------------------
    def build_edge(last):
        nc = new_nc()
        table = nc.declare_dram_parameter("table", [TROWS, ROWF], dt.float32, False)
        gidx = nc.declare_dram_parameter("gidx", [128, NIDX16], dt.int16, False)
        dstrel = nc.declare_dram_parameter("dstrel", [128, CH], dt.bfloat16, False)
        NH = 1 if last else 4
        HC = 64 if last else 128
        MC = HC + NH
        C = HC // NH
        esum = nc.declare_dram_parameter("esum", [128, CH * NH], dt.float32, False)
        iota = nc.declare_dram_parameter("iota", [128, 128], dt.bfloat16, False)
        bias = nc.declare_dram_parameter("bias", [128, HC], dt.float32, False)
        if last:
            out = nc.declare_dram_parameter("out", [NPC, HC], dt.float32, True)
        else:
            ident = nc.declare_dram_parameter("ident", [128, 128], dt.float32, False)
            wnext = nc.declare_dram_parameter("wext", [128, 136], dt.bfloat16, False)
            out_h = nc.declare_dram_parameter("slab_h", [NPC, ROW], dt.bfloat16, True)
            out_a = nc.declare_dram_parameter("slab_a", [NPC, 8], dt.float32, True)

        with tile.TileContext(nc) as tc:
            with tc.tile_pool(name="c", bufs=1) as cpool, \
                 tc.tile_pool(name="g", bufs=3) as gpool, \
                 tc.tile_pool(name="w", bufs=3) as wpool, \
                 tc.tile_pool(name="ps", bufs=2, space="PSUM") as pps, \
                 tc.tile_pool(name="ps2", bufs=2, space="PSUM") as pps2:
                regs = {}
                for ni in meta["calls"]:
                    if ni not in regs:
                        regs[ni] = nc.gpsimd.to_reg(ni)
                iot = cpool.tile([128, 128], dt.bfloat16, tag="iota")
                nc.sync.dma_start(out=iot[:], in_=iota[:])
                bia = cpool.tile([128, HC], dt.float32, tag="bias")
                nc.sync.dma_start(out=bia[:], in_=bias[:])
                idxs = cpool.tile([128, NIDX16], dt.int16, tag="gidx")
                nc.sync.dma_start(out=idxs[:], in_=gidx[:])
                drel = cpool.tile([128, CH], dt.bfloat16, tag="drel")
                nc.sync.dma_start(out=drel[:], in_=dstrel[:])
                esu = cpool.tile([128, CH * NH], dt.float32, tag="esum")
                nc.sync.dma_start(out=esu[:], in_=esum[:])
                if not last:
                    idn = cpool.tile([128, 128], dt.float32, tag="ident")
                    nc.sync.dma_start(out=idn[:], in_=ident[:])
                    wnx = cpool.tile([128, 136], dt.bfloat16, tag="wext")
                    nc.sync.dma_start(out=wnx[:], in_=wnext[:])

                tab_lo = table[0:LO_LIM, :]
                tab_hi = table[HI_OFF:TROWS, :]
                ioff = 0
                qn = 0
                ci = 0          # call index
                cb = 0          # chunk base
                for gs in groups:
                    kls = [int(KL[s]) for s in gs]
                    khs = [int(KH[s]) for s in gs]
                    KG = sum(kls) + sum(khs)
                    G = gpool.tile([128, KG, ROWF], dt.float32, tag="G")
                    Gb = G[:].bitcast(dt.bfloat16)      # [128, KG, 128]
                    k0 = 0
                    for hf, kk in ((0, sum(kls)), (1, sum(khs))):
                        for o in range(0, kk, MAXCH):
                            kc = min(MAXCH, kk - o)
                            ni = kc * 128
                            assert ni == meta["calls"][ci]
                            nc.gpsimd.dma_gather(
                                G[:, k0:k0 + kc, :],
                                tab_lo if hf == 0 else tab_hi,
                                idxs[:, ioff:ioff + ni // 16],
                                num_idxs=ni, num_idxs_reg=regs[ni],
                                elem_size=ROWF, queue_num=qn,
                                single_packet=False)
                            qn = (qn + 1) % 4
                            ioff += ni // 16
                            k0 += kc
                            ci += 1

                    # w = exp(lrelu(esum)); padding slots underflow to 0
                    wv = wpool.tile([128, KG * NH], dt.float32, tag="wv")
                    nc.scalar.activation(wv[:], esu[:, cb * NH:(cb + KG) * NH],
                                         AF.Prelu, alpha=NEG)
                    nc.scalar.activation(wv[:], wv[:], AF.Exp)

                    # one-hot(dst_rel) [128, KG, 128] bf16
                    oh = wpool.tile([128, KG * 128], dt.bfloat16, tag="oh")
                    nc.vector.tensor_tensor(
                        oh[:].rearrange("p (k j) -> p k j", j=128),
                        drel[:, cb:cb + KG]
                            .rearrange("p (k o) -> p k o", o=1)
                            .to_broadcast([128, KG, 128]),
                        iot[:].rearrange("p (o j) -> p o j", o=1)
                            .to_broadcast([128, KG, 128]),
                        op=mybir.AluOpType.is_equal)

                    # M = [h*w | w] bf16; w lands in M's tail columns via the
                    # scalar-engine bf16 cast and doubles as the mul operand
                    M = wpool.tile([128, KG * MC], dt.bfloat16, tag="M")
                    Mv = M[:].rearrange("p (k m) -> p k m", m=MC)
                    nc.scalar.activation(
                        Mv[:, :, HC:MC],
                        wv[:].rearrange("p (k h) -> p k h", h=NH), AF.Copy)
                    nc.vector.tensor_mul(
                        Mv[:, :, 0:HC].rearrange("p k (h c) -> p k h c", c=C),
                        Gb[:, :, 0:HC].rearrange("p k (h c) -> p k h c", c=C),
                        Mv[:, :, HC:MC].rearrange("p k (h o) -> p k h o", o=1)
                            .to_broadcast([128, KG, NH, C]))

                    # per-slot scatter-accumulate + epilogue
                    ng = len(gs)
                    if last:
                        oxp = wpool.tile([128, ng, HC], dt.float32, tag="oxp")
                    else:
                        growh = wpool.tile([128, ng, ROW], dt.bfloat16,
                                           tag="growh")
                        growa = wpool.tile([128, ng, 8], dt.float32, tag="growa")
                    kbase_lo = 0
                    kbase_hi = sum(kls)
                    for i, s in enumerate(gs):
                        chunks = (list(range(kbase_lo, kbase_lo + kls[i]))
                                  + list(range(kbase_hi, kbase_hi + khs[i])))
                        kbase_lo += kls[i]
                        kbase_hi += khs[i]
                        T = pps.tile([128, MC], dt.float32, tag=f"acc{i}")
                        for j, k in enumerate(chunks):
                            nc.tensor.matmul(T[:],
                                             lhsT=oh[:, k * 128:(k + 1) * 128],
                                             rhs=Mv[:, k, :],
                                             start=(j == 0),
                                             stop=(j == len(chunks) - 1))
                        rcp = wpool.tile([128, NH], dt.float32, tag=f"rcp{i}")
                        nc.vector.reciprocal(rcp[:], T[:, HC:MC])
                        xp = wpool.tile([128, HC], dt.float32, tag=f"xp{i}")
                        nc.vector.tensor_mul(
                            xp[:].rearrange("p (h c) -> p h c", c=C),
                            T[:, 0:HC].rearrange("p (h c) -> p h c", c=C),
                            rcp[:].rearrange("p (h o) -> p h o", o=1)
                                .to_broadcast([128, NH, C]))
                        nc.vector.tensor_add(xp[:], xp[:], bia[:])
                        if last:
                            nc.scalar.activation(oxp[:, i, :], xp[:],
                                                 AF.Prelu, alpha=NEG)
                        else:
                            nc.scalar.activation(xp[:], xp[:], AF.Prelu,
                                                 alpha=NEG)
                            pt = pps2.tile([128, 128], dt.float32, tag="xt")
                            nc.tensor.transpose(out=pt[:], in_=xp[:],
                                                identity=idn[:])
                            xt = wpool.tile([128, 128], dt.bfloat16,
                                            tag=f"xts{i}")
                            nc.scalar.activation(xt[:], pt[:], AF.Copy)
                            ph = pps2.tile([128, 136], dt.float32, tag="h2")
                            nc.tensor.matmul(ph[:], lhsT=xt[:], rhs=wnx[:],
                                             start=True, stop=True)
                            nc.scalar.activation(growh[:, i, :], ph[:, 0:128],
                                                 AF.Copy)
                            nc.vector.tensor_copy(growa[:, i, :], ph[:, 128:136])
                    s0 = gs[0]
                    s1 = gs[-1] + 1
                    if last:
                        nc.sync.dma_start(
                            out=out[s0 * 128:s1 * 128, :]
                                .rearrange("(b p) r -> p b r", p=128),
                            in_=oxp[:])
                    else:
                        nc.sync.dma_start(
                            out=out_h[s0 * 128:s1 * 128, :]
                                .rearrange("(b p) r -> p b r", p=128),
                            in_=growh[:])
                        nc.sync.dma_start(
                            out=out_a[s0 * 128:s1 * 128, :]
                                .rearrange("(b p) r -> p b r", p=128),
                            in_=growa[:])
                    cb += KG
        _split_multiwait_ctrl(nc)
        nc.compile()
        return nc

    return nc0, build_edge(False), build_edge(True)


# --------------------------------------------------------------------------
# entry point
# --------------------------------------------------------------------------
def kernel(x, edge_index, W0, as0, ad0, b0, W1, as1, ad1, b1, W2, as2, ad2, b2):
    _install_ntff_hook()
    from concourse.bass_utils import run_bass_kernel_spmd

    x = np.asarray(x, np.float32)
    meta = _prep_graph(np.asarray(edge_index))
    meta["src_row"] = meta["rowof"][meta["src"]]
    meta["dst_row"] = meta["rowof"][meta["dst"]]
    nc0, nc12, nc3 = _get_kernels(meta)
    cores = list(range(NCORES))
    trace = bool(os.environ.get("BASS_TRACE"))

    iota = np.tile(np.arange(128, dtype=np.float32), (128, 1)).astype(BF16)
    ident = np.eye(128, dtype=np.float32)
    w0e, w1e, w2e = _wext(W0, as0, ad0), _wext(W1, as1, ad1), _wext(W2, as2, ad2)
    rowof = meta["rowof"]
    garbage_rows = rowof[N:]

    total_ns = [0]

    def run(nc, maps):
        last = None
        for attempt in range(3):
            try:
                r = run_bass_kernel_spmd(nc, maps, core_ids=cores, trace=trace)
                if r.exec_time_ns:
                    total_ns[0] += int(r.exec_time_ns)
                    if os.environ.get("KERNEL_VERBOSE"):
                        print(f"[launch] exec={r.exec_time_ns}ns", file=sys.stderr)
                return r.results
            except Exception as e:  # intermittent NRT exec-unit crashes
                last = e
        raise last

    def assemble(res):
        th = np.zeros((TROWS, ROW), BF16)
        ar = np.zeros((TROWS, 8), np.float32)
        th[1:NPAD + 1] = np.concatenate(
            [np.asarray(res[c]["slab_h"]) for c in cores], axis=0)
        ar[1:NPAD + 1] = np.concatenate(
            [np.asarray(res[c]["slab_a"]) for c in cores], axis=0)
        th[garbage_rows] = BF16(0)
        ar[garbage_rows] = 0.0
        return np.ascontiguousarray(th).view(np.float32), ar

    # dense0: x rows permuted into table order
    perm_nodes = np.empty(NPAD, np.int64)
    perm_nodes[rowof - 1] = np.arange(NPAD)
    xpad = np.zeros((NPAD, 128), np.float32)
    xpad[:N] = x
    xTb = np.ascontiguousarray(xpad[perm_nodes].T.astype(BF16))
    res = run(nc0, [{"xT": np.ascontiguousarray(xTb[:, c * NPC:(c + 1) * NPC]),
                     "wext": w0e} for c in cores])
    table, arows = assemble(res)

    def edge_maps(tab, ar, wnext, bias_vec, hc, nh):
        bias = np.tile(np.asarray(bias_vec, np.float32)[:hc], (128, 1))
        maps = []
        for c in cores:
            pc = meta["per_core"][c]
            m = {"table": tab, "gidx": pc["gidx"], "dstrel": pc["dstrel"],
                 "esum": _esum_stream(meta, c, ar, nh),
                 "iota": iota, "bias": bias}
            if wnext is not None:
                m["ident"] = ident
                m["wext"] = wnext
            maps.append(m)
        return maps

    res = run(nc12, edge_maps(table, arows, w1e, b0, 128, 4))
    table, arows = assemble(res)
    res = run(nc12, edge_maps(table, arows, w2e, b1, 128, 4))
    table, arows = assemble(res)
    res = run(nc3, edge_maps(table, arows, None, b2, 64, 1))
    outp = np.concatenate([np.asarray(res[c]["out"]) for c in cores], axis=0)
    out = np.empty((NPAD, 64), np.float32)
    out[perm_nodes] = outp
    kernel.last_exec_ns = total_ns[0]
    return np.ascontiguousarray(out[:N], dtype=np.float32)
